# revision 1
# baseline (speedup 1.0000x reference)
"""Trainium2 Bass kernel for a 2-layer GCN (nn_CorrelationGNN).

Math (reference):
    src,dst = edges + self loops;  deg over dst;  dinv = deg^-1/2
    h1 = relu(S @ (x @ W0) + b0),  S = D^-1/2 (A+I) D^-1/2
    h2 = relu(S @ (h1 @ W1) + b1)
    out = h2 @ Wf + bf

Factorization used: S @ (h W) = dinv * Agg(dinv * h) @ W, where Agg is the
pure 0/1 adjacency gather-sum (S commutes with the feature matmul).

Distribution: destination nodes sharded across 8 cores (12500/core, padded
to 12544 = 128*98).  Ranks are degree-sorted; rank r -> (p=r%128, g=r//128),
table row within a core slice = p*98+g.  Gather source is an fp16 table
[100352, 128] (row = 32 feats + 96 zeros = 256B) assembled per core from an
AllGather of compact fp16 slices.  Edges are gathered with gpsimd dma_gather
(int16 idxs, 4 SWDGE queues, <=1024 idxs/inst) as 4 source-quarter streams;
per (quarter, g-column) the slot count K is the max over all cores so the
traced program is identical on every core (SPMD).
"""

import numpy as np

import concourse.bass as bass  # noqa: F401
import concourse.bacc as bacc
import concourse.mybir as mybir
from concourse.tile import TileContext
from concourse.bass_utils import run_bass_kernel_spmd

P = 128
N = 100000
F = 32
NPC = 12500          # real nodes per core
G = 98               # g-columns per core
NPCP = P * G         # padded nodes per core = 12544
NROWS = 8 * NPCP     # global table rows = 100352
QROWS = NROWS // 4   # 25088, int16-addressable quarter
QZREL = 12543        # guaranteed-zero pad row, same offset in every quarter
KCAP = 8             # slots per dma_gather inst (8*128 = 1024 idx cap)
FDT = mybir.dt.float32
HDT = mybir.dt.float16


def _build_plan_and_offsets(edge_index):
    src = np.asarray(edge_index[0], dtype=np.int64)
    dst = np.asarray(edge_index[1], dtype=np.int64)
    loops = np.arange(N, dtype=np.int64)
    src = np.concatenate([src, loops])
    dst = np.concatenate([dst, loops])

    deg = np.bincount(dst, minlength=N).astype(np.float64)
    dinv = (1.0 / np.sqrt(deg)).astype(np.float32)

    node_core = np.arange(N) // NPC
    rank = np.empty(N, dtype=np.int64)
    perms = []
    for c in range(8):
        nodes = np.arange(c * NPC, (c + 1) * NPC)
        order = np.argsort(-deg[nodes], kind="stable")
        perm = nodes[order]
        perms.append(perm)
        rank[perm] = np.arange(NPC)
    trow = node_core * NPCP + (rank % P) * G + (rank // P)
    quarter = trow // QROWS
    qrel = (trow % QROWS).astype(np.int32)

    # per-core edges sorted by (dst rank, src quarter); self-loops handled
    # on-device as agg init = xd_own, so drop them from the token streams
    noloop = src != dst
    srcn, dstn = src[noloop], dst[noloop]
    edges = []
    cnt_rq = np.zeros((8, NPC * 4), dtype=np.int32)
    for c in range(8):
        m = (dstn // NPC) == c
        s, d = srcn[m], dstn[m]
        key = rank[d] * 4 + quarter[s]
        order = np.argsort(key, kind="stable")
        edges.append((qrel[s][order], key[order]))
        cnt_rq[c] = np.bincount(key, minlength=NPC * 4)

    crq = cnt_rq.reshape(8, NPC, 4)
    K = np.zeros((G, 4), dtype=np.int32)
    for g in range(G):
        K[g] = crq[:, g * P : (g + 1) * P, :].max(axis=(0, 1))

    # shared instruction plan: (q, g, k0, kc, col0)
    plan = []
    col = 0
    for q in range(4):
        for g in range(G):
            k0 = 0
            while k0 < int(K[g, q]):
                kc = min(KCAP, int(K[g, q]) - k0)
                plan.append((q, g, k0, kc, col))
                col += kc * P // 16
                k0 += kc
    totc = col

    # zero pad rows (pad ranks 12500.. of the two cores in each quarter have
    # dinv=0 so their table rows are always zero); spread pad tokens across
    # them to avoid HBM hot-spotting on a single row.
    pad_ranks = np.arange(NPC, NPCP)
    zrel = (pad_ranks % P) * G + (pad_ranks // P)  # within-slice rows
    zero_rows = np.concatenate([zrel, zrel + NPCP]).astype(np.int16)  # both cores

    # per-core offset tables [16, totc], idx i of inst at [i%16, col0+i//16]
    offs_all = []
    for c in range(8):
        qr, key = edges[c]
        cnt = np.bincount(key, minlength=NPC * 4)
        ptr = np.zeros(NPC * 4 + 1, dtype=np.int64)
        np.cumsum(cnt, out=ptr[1:])
        rngpad = np.random.default_rng(c)
        offs = zero_rows[rngpad.integers(0, len(zero_rows), size=totc * 16)].astype(
            np.int16
        )
        for (q, g, k0, kc, col0) in plan:
            base = col0 * 16
            lo = g * P
            nreal = min(P, NPC - lo)
            # vectorized over p
            ps = np.arange(nreal)
            rk = lo + ps
            a = ptr[rk * 4 + q]
            b = ptr[rk * 4 + q + 1]
            for kk in range(kc):
                k = k0 + kk
                sel = (a + k) < b
                pos = base + kk * P + ps[sel]
                offs[pos] = qr[(a + k)[sel]]
        offs_all.append(offs.reshape(totc, 16).T.copy())

    return plan, totc, dinv, perms, offs_all


def _build_program(plan, totc):
    nc = bacc.Bacc(
        "TRN2", target_bir_lowering=False, debug=False, num_devices=8,
        num_swdge_queues=4,
    )
    x_own = nc.declare_dram_parameter("x_own", [P, G * F], FDT, isOutput=False)
    dinv_own = nc.declare_dram_parameter("dinv_own", [P, G], FDT, isOutput=False)
    offs = nc.declare_dram_parameter("offs", [P, totc], mybir.dt.int16, isOutput=False)
    W0 = nc.declare_dram_parameter("W0", [F, F], FDT, isOutput=False)
    W1 = nc.declare_dram_parameter("W1", [F, F], FDT, isOutput=False)
    Wf = nc.declare_dram_parameter("Wf", [F, F], FDT, isOutput=False)
    b0 = nc.declare_dram_parameter("b0", [F, 1], FDT, isOutput=False)
    b1 = nc.declare_dram_parameter("b1", [F, 1], FDT, isOutput=False)
    bf = nc.declare_dram_parameter("bf", [F, 1], FDT, isOutput=False)
    out_own = nc.declare_dram_parameter("out_own", [P, G * F], FDT, isOutput=True)

    cc_in = nc.dram_tensor("cc_in", [NPCP, F], HDT)
    cc_out = nc.dram_tensor("cc_out", [NROWS, F], HDT, addr_space="Shared")
    table = nc.dram_tensor("table", [NROWS, 4 * F], HDT)

    from concourse.masks import make_identity

    with TileContext(nc) as tc:
        with (
            tc.tile_pool(name="persist", bufs=1) as pp,
            tc.tile_pool(name="gpool", bufs=10) as gp,
            tc.tile_pool(name="spool", bufs=4) as sp,
            tc.tile_pool(name="psum", bufs=2, space="PSUM") as psp,
        ):
            offs_t = pp.tile([P, totc], mybir.dt.int16)
            nc.sync.dma_start(out=offs_t[:], in_=offs[:])
            dinv_t = pp.tile([P, G], FDT)
            nc.sync.dma_start(out=dinv_t[:], in_=dinv_own[:])
            w0_t = pp.tile([F, F], FDT)
            nc.sync.dma_start(out=w0_t[:], in_=W0[:])
            w1_t = pp.tile([F, F], FDT)
            nc.sync.dma_start(out=w1_t[:], in_=W1[:])
            wf_t = pp.tile([F, F], FDT)
            nc.sync.dma_start(out=wf_t[:], in_=Wf[:])
            b0_t = pp.tile([F, 1], FDT)
            nc.sync.dma_start(out=b0_t[:], in_=b0[:])
            b1_t = pp.tile([F, 1], FDT)
            nc.sync.dma_start(out=b1_t[:], in_=b1[:])
            bf_t = pp.tile([F, 1], FDT)
            nc.sync.dma_start(out=bf_t[:], in_=bf[:])
            ident = pp.tile([P, P], FDT)
            make_identity(nc, ident[:])

            xcur = pp.tile([P, G * F], FDT, tag="xcur")
            nc.sync.dma_start(out=xcur[:], in_=x_own[:])
            agg = pp.tile([P, G * F], FDT, tag="agg")
            xd_own = pp.tile([P, G * F], HDT, tag="xdown")

            dinv_b = dinv_t[:].to_broadcast([P, G, F])

            def scale_to_table(src_tile, scope):
                with nc.named_scope(scope):
                    nc.vector.tensor_tensor(
                        out=xd_own[:].rearrange("p (g f) -> p g f", f=F),
                        in0=src_tile[:].rearrange("p (g f) -> p g f", f=F),
                        in1=dinv_b,
                        op=mybir.AluOpType.mult,
                    )
                    nc.sync.dma_start(out=cc_in[:], in_=xd_own[:])
                    nc.gpsimd.collective_compute(
                        "AllGather",
                        mybir.AluOpType.bypass,
                        replica_groups=[list(range(8))],
                        ins=[cc_in[:]],
                        outs=[cc_out[:]],
                    )
                    for qq in range(4):
                        nc.sync.dma_start(
                            out=table[qq * QROWS : (qq + 1) * QROWS, :F],
                            in_=cc_out[qq * QROWS : (qq + 1) * QROWS, :],
                        )

            def gather_layer(scope):
                with nc.named_scope(scope):
                    # self-loop contribution: agg starts at xd_own
                    nc.vector.tensor_copy(out=agg[:], in_=xd_own[:])
                    for (q, g, k0, kc, col0) in plan:
                        gt = gp.tile([P, KCAP, 4 * F], HDT, tag="g")
                        nc.gpsimd.dma_gather(
                            out_ap=gt[:, :kc, :],
                            in_ap=table[q * QROWS : (q + 1) * QROWS, :],
                            idxs_ap=offs_t[:, col0 : col0 + kc * P // 16],
                            num_idxs=kc * P,
                            num_idxs_reg=kc * P,
                            elem_size=4 * F,
                            queue_num=(q * G + g) % 4,
                        )
                        if kc == 1:
                            nc.vector.tensor_add(
                                out=agg[:, g * F : (g + 1) * F],
                                in0=agg[:, g * F : (g + 1) * F],
                                in1=gt[:, 0, :F],
                            )
                        else:
                            # fp16 pairs added into f32 (no fp16 accumulation)
                            h2 = kc // 2
                            h = (kc + 1) // 2
                            red = sp.tile([P, 4, F], FDT, tag="red")
                            nc.vector.tensor_add(
                                out=red[:, :h2, :],
                                in0=gt[:, 0 : 2 * h2 : 2, :F],
                                in1=gt[:, 1 : 2 * h2 : 2, :F],
                            )
                            if kc % 2:
                                nc.vector.tensor_copy(
                                    out=red[:, h2, :], in_=gt[:, kc - 1, :F]
                                )
                            if h == 1:
                                nc.vector.tensor_add(
                                    out=agg[:, g * F : (g + 1) * F],
                                    in0=agg[:, g * F : (g + 1) * F],
                                    in1=red[:, 0, :],
                                )
                            else:
                                red2 = sp.tile([P, F], FDT, tag="red2")
                                nc.vector.reduce_sum(
                                    out=red2[:],
                                    in_=red[:, :h, :].rearrange("p k f -> p f k"),
                                    axis=mybir.AxisListType.X,
                                )
                                nc.vector.tensor_add(
                                    out=agg[:, g * F : (g + 1) * F],
                                    in0=agg[:, g * F : (g + 1) * F],
                                    in1=red2[:],
                                )

            def layer_tail(W_t, bias_t, relu, dest, scope, W2_t=None, bias2_t=None):
                with nc.named_scope(scope):
                    nc.vector.tensor_tensor(
                        out=agg[:].rearrange("p (g f) -> p g f", f=F),
                        in0=agg[:].rearrange("p (g f) -> p g f", f=F),
                        in1=dinv_b,
                        op=mybir.AluOpType.mult,
                    )
                    for g in range(G):
                        ps1 = psp.tile([F, P], FDT, tag="ps1")
                        nc.tensor.matmul(
                            out=ps1[:], lhsT=agg[:, g * F : (g + 1) * F], rhs=ident[:],
                            start=True, stop=True,
                        )
                        s1 = sp.tile([F, P], FDT, tag="s1")
                        nc.vector.tensor_copy(out=s1[:], in_=ps1[:])
                        ps2 = psp.tile([F, P], FDT, tag="ps2")
                        nc.tensor.matmul(out=ps2[:], lhsT=W_t[:], rhs=s1[:], start=True, stop=True)
                        s2 = sp.tile([F, P], FDT, tag="s2")
                        if relu:
                            nc.scalar.activation(
                                out=s2[:], in_=ps2[:],
                                func=mybir.ActivationFunctionType.Relu,
                                bias=b0_t[:, :1] if bias_t is b0_t else bias_t[:, :1],
                                scale=1.0,
                            )
                        else:
                            nc.vector.tensor_scalar(
                                out=s2[:], in0=ps2[:], scalar1=bias_t[:, :1],
                                scalar2=None, op0=mybir.AluOpType.add,
                            )
                        if W2_t is not None:
                            ps3 = psp.tile([F, P], FDT, tag="ps3")
                            nc.tensor.matmul(out=ps3[:], lhsT=W2_t[:], rhs=s2[:], start=True, stop=True)
                            s2b = sp.tile([F, P], FDT, tag="s2b")
                            nc.vector.tensor_scalar(
                                out=s2b[:], in0=ps3[:], scalar1=bias2_t[:, :1],
                                scalar2=None, op0=mybir.AluOpType.add,
                            )
                            s2 = s2b
                        psb = psp.tile([P, F], FDT, tag="psb")
                        nc.tensor.matmul(
                            out=psb[:], lhsT=s2[:], rhs=ident[:F, :F], start=True, stop=True
                        )
                        nc.vector.tensor_copy(out=dest[:, g * F : (g + 1) * F], in_=psb[:])

            scale_to_table(xcur, "table0")
            gather_layer("gather0")
            layer_tail(w0_t, b0_t, relu=True, dest=xcur, scope="tail0")
            scale_to_table(xcur, "table1")
            gather_layer("gather1")
            outt = pp.tile([P, G * F], FDT, tag="outt")
            layer_tail(
                w1_t, b1_t, relu=True, dest=outt, scope="tail1", W2_t=wf_t, bias2_t=bf_t
            )
            nc.sync.dma_start(out=out_own[:], in_=outt[:])

    nc.compile()
    return nc


_CACHE = {}


def kernel(x, edge_index, W0, b0, W1, b1, Wf, bf):
    x = np.asarray(x, dtype=np.float32)
    edge_index = np.asarray(edge_index)
    plan, totc, dinv, perms, offs_all = _build_plan_and_offsets(edge_index)

    key = ("prog", totc, len(plan))
    if key not in _CACHE:
        _CACHE[key] = _build_program(plan, totc)
    nc = _CACHE[key]

    in_maps = []
    rr = np.arange(NPC)
    pp_, gg = rr % P, rr // P
    for c in range(8):
        perm = perms[c]
        xo = np.zeros((P, G, F), dtype=np.float32)
        dv = np.zeros((P, G), dtype=np.float32)
        xo[pp_, gg, :] = x[perm]
        dv[pp_, gg] = dinv[perm]
        in_maps.append(
            {
                "x_own": xo.reshape(P, G * F),
                "dinv_own": dv,
                "offs": np.tile(offs_all[c], (8, 1)).astype(np.int16),
                "W0": np.asarray(W0, np.float32),
                "W1": np.asarray(W1, np.float32),
                "Wf": np.asarray(Wf, np.float32),
                "b0": np.asarray(b0, np.float32).reshape(F, 1),
                "b1": np.asarray(b1, np.float32).reshape(F, 1),
                "bf": np.asarray(bf, np.float32).reshape(F, 1),
            }
        )

    res = run_bass_kernel_spmd(nc, in_maps, list(range(8)))
    kernel._last_results = res

    out = np.zeros((N, F), dtype=np.float32)
    for c in range(8):
        oo = res.results[c]["out_own"].reshape(P, G, F)
        out[perms[c]] = oo[pp_, gg, :]
    return out



# revision 5
# speedup vs baseline: 1.3784x; 1.3784x over previous
"""Trainium2 Bass kernel for a 2-layer GCN (nn_CorrelationGNN).

Math (reference):
    src,dst = edges + self loops;  deg over dst;  dinv = deg^-1/2
    h1 = relu(S @ (x @ W0) + b0),  S = D^-1/2 (A+I) D^-1/2
    h2 = relu(S @ (h1 @ W1) + b1)
    out = h2 @ Wf + bf

Factorization: S@(h W) = (D^-1/2 Agg (D^-1/2 h)) W.  Source-side dinv is
pre-scaled into the fp16 gather table; dst-side dinv is folded into the
segment-selection matrices.

Distribution: dst nodes sharded across 8 cores (12500/core).  rank = local
node id; dst tile t = ranks [128t,128t+128), partition p = rank%128.  Table
row (per core slice) = p*G + t, rows are 256B (32 fp16 feats + pad) because
dma_gather elems must be 256B-multiples.

Aggregation: edges sorted by (src-quarter, dst rank); per (quarter q, tile
t) the segment is padded to the cross-core max rounded to 128 so all cores
trace an identical program.  Gathers run 4096 idxs/instruction on 4 SWDGE
queues.  Each 128-token chunk is reduced with one tensor-engine matmul
psum[f, dstp] += gt[:, slot, :32].T @ S where S[tok, dstp] =
(dstp_tok == dstp) * dinv_dst, built on-chip by a fused is_equal+mult
tensor_scalar against an iota tile.  Self loops are one diagonal matmul
per tile: psum += xd_tile.T @ diag(dinv).  Tails stay feature-major so
W0/W1/Wf apply as single 32-contraction matmuls; one identity matmul per
tile transposes back to node-major.
"""

import numpy as np

import concourse.bass as bass  # noqa: F401
import concourse.bacc as bacc
import concourse.mybir as mybir
from concourse.tile import TileContext
from concourse.bass_utils import run_bass_kernel_spmd

P = 128
N = 100000
F = 32
NPC = 12500          # real nodes per core
G = 98               # dst tiles per core (12544/128)
NPCP = P * G         # padded nodes per core = 12544
NROWS = 8 * NPCP     # global table rows = 100352
QROWS = NROWS // 4   # 25088, int16-addressable quarter
NIDX = 1024          # idxs per dma_gather instruction (HW cap)
CPI = NIDX // P      # chunks (matmuls) per gather instruction = 32
GT_BUFS = 6
FDT = mybir.dt.float32
HDT = mybir.dt.float16


def _build_plan(edge_index):
    src = np.asarray(edge_index[0], dtype=np.int64)
    dst = np.asarray(edge_index[1], dtype=np.int64)

    deg = (np.bincount(dst, minlength=N) + 1).astype(np.float64)  # + self loop
    dinv = (1.0 / np.sqrt(deg)).astype(np.float32)

    rank = np.arange(N, dtype=np.int64) % NPC
    trow = (np.arange(N) // NPC) * NPCP + (rank % P) * G + (rank // P)
    quarter = (trow // QROWS).astype(np.int64)
    qrel = (trow % QROWS).astype(np.int16)

    keep = src != dst  # self loops handled as diagonal matmuls
    es, ed = src[keep], dst[keep]
    ecore = ed // NPC
    eq = quarter[es]
    erank = ed % NPC
    etile = erank // P

    # global sort by (core, quarter, dst rank)
    key = (((ecore * 4 + eq) << 14) | erank).astype(np.int64)
    order = np.argsort(key, kind="stable")
    es, ed, ecore, eq, erank, etile = (
        a[order] for a in (es, ed, ecore, eq, erank, etile)
    )

    # per (core, q, tile) counts -> shared padded segment lengths
    gid = (ecore * 4 + eq) * G + etile
    cnt = np.bincount(gid, minlength=8 * 4 * G).reshape(8, 4, G)
    L = ((cnt.max(axis=0) + P - 1) // P) * P          # [4, G]
    segchunks = L // P                                # [4, G]

    tok_base = np.zeros((4, G), dtype=np.int64)       # 128-aligned stream pos
    for q in range(4):
        tok_base[q, 1:] = np.cumsum(L[q])[:-1]
    Tq = L.sum(axis=1)                                # tokens per stream
    Tq_pad = ((Tq + NIDX - 1) // NIDX) * NIDX
    ninst = (Tq_pad // NIDX).astype(np.int64)
    stream_col_base = np.zeros(4, dtype=np.int64)     # offs col base per q
    stream_col_base[1:] = np.cumsum(Tq_pad // 16)[:3]
    OFFC = int((Tq_pad // 16).sum())

    # dstp/dinv column index per (q, t): t-major, q inner, chunk innermost
    col_index = np.zeros((4, G), dtype=np.int64)
    c = 0
    for t in range(G):
        for q in range(4):
            col_index[q, t] = c
            c += int(segchunks[q, t])
    NCOLS = c

    # per-edge position within its (core,q,t) group
    gstart = np.zeros(8 * 4 * G + 1, dtype=np.int64)
    np.cumsum(cnt.reshape(-1), out=gstart[1:])
    within = np.arange(len(es)) - gstart[gid]

    plan = dict(
        L=L, segchunks=segchunks, tok_base=tok_base, ninst=ninst,
        stream_col_base=stream_col_base, OFFC=OFFC, col_index=col_index,
        NCOLS=NCOLS,
    )

    # per-core input tables
    offs_all, dstp_all, dinvc_all = [], [], []
    for cidx in range(8):
        m = ecore == cidx
        q_, rank_, tile_, w_ = eq[m], erank[m], etile[m], within[m]
        s_, d_ = es[m], ed[m]
        pos = tok_base[q_, tile_] + w_                 # stream position
        offs = np.zeros((16, OFFC), dtype=np.int16)
        offs[pos % 16, stream_col_base[q_] + pos // 16] = qrel[s_]
        dstp = np.full((NCOLS, P), -1.0, dtype=np.float32)
        dinvc = np.zeros((NCOLS, P), dtype=np.float32)
        colpos = col_index[q_, tile_] + w_ // P
        dstp[colpos, w_ % P] = (rank_ - tile_ * P).astype(np.float32)
        dinvc[colpos, w_ % P] = dinv[d_]
        offs_all.append(offs)
        dstp_all.append(dstp.T.copy())
        dinvc_all.append(dinvc.T.copy())

    return plan, dinv, offs_all, dstp_all, dinvc_all


def _build_program(plan):
    L = plan["L"]; segchunks = plan["segchunks"]; tok_base = plan["tok_base"]
    ninst = plan["ninst"]; scb = plan["stream_col_base"]
    OFFC = plan["OFFC"]; col_index = plan["col_index"]; NCOLS = plan["NCOLS"]

    nc = bacc.Bacc(
        "TRN2", target_bir_lowering=False, debug=False, num_devices=8,
        num_swdge_queues=4,
    )
    table0 = nc.declare_dram_parameter("table0", [NROWS, 4 * F], HDT, isOutput=False)
    xd_own = nc.declare_dram_parameter("xd_own", [P, G * F], HDT, isOutput=False)
    dinv_own = nc.declare_dram_parameter("dinv_own", [P, G], FDT, isOutput=False)
    offs = nc.declare_dram_parameter("offs", [P, OFFC], mybir.dt.int16, isOutput=False)
    dstp = nc.declare_dram_parameter("dstp", [P, NCOLS], FDT, isOutput=False)
    dinvc = nc.declare_dram_parameter("dinvc", [P, NCOLS], FDT, isOutput=False)
    W0 = nc.declare_dram_parameter("W0", [F, F], FDT, isOutput=False)
    W1 = nc.declare_dram_parameter("W1", [F, F], FDT, isOutput=False)
    Wf = nc.declare_dram_parameter("Wf", [F, F], FDT, isOutput=False)
    b0 = nc.declare_dram_parameter("b0", [F, 1], FDT, isOutput=False)
    b1 = nc.declare_dram_parameter("b1", [F, 1], FDT, isOutput=False)
    bf = nc.declare_dram_parameter("bf", [F, 1], FDT, isOutput=False)
    out_own = nc.declare_dram_parameter("out_own", [P, G * F], FDT, isOutput=True)

    cc_in = nc.dram_tensor("cc_in", [NPCP, F], HDT)
    cc_out = nc.dram_tensor("cc_out", [NROWS, F], HDT, addr_space="Shared")
    table1 = nc.dram_tensor("table1", [NROWS, 4 * F], HDT)

    from concourse.masks import make_identity

    with TileContext(nc) as tc:
        with (
            tc.tile_pool(name="persist", bufs=1) as pp,
            tc.tile_pool(name="g0", bufs=GT_BUFS) as gp0,
            tc.tile_pool(name="g1", bufs=GT_BUFS) as gp1,
            tc.tile_pool(name="g2", bufs=GT_BUFS) as gp2,
            tc.tile_pool(name="g3", bufs=GT_BUFS) as gp3,
            tc.tile_pool(name="spool", bufs=8) as sp,
            tc.tile_pool(name="fmpool", bufs=6) as fmp,
            tc.tile_pool(name="psum", bufs=2, space="PSUM") as psp,
        ):
            gpools = [gp0, gp1, gp2, gp3]
            offs_t = pp.tile([P, OFFC], mybir.dt.int16)
            nc.sync.dma_start(out=offs_t[:], in_=offs[:])
            dstp_t = pp.tile([P, NCOLS], FDT)
            nc.sync.dma_start(out=dstp_t[:], in_=dstp[:])
            dinvc_t = pp.tile([P, NCOLS], FDT)
            nc.sync.dma_start(out=dinvc_t[:], in_=dinvc[:])
            dinv_t = pp.tile([P, G], FDT)
            nc.sync.dma_start(out=dinv_t[:], in_=dinv_own[:])
            w0_t = pp.tile([F, F], FDT)
            nc.sync.dma_start(out=w0_t[:], in_=W0[:])
            w1_t = pp.tile([F, F], FDT)
            nc.sync.dma_start(out=w1_t[:], in_=W1[:])
            wf_t = pp.tile([F, F], FDT)
            nc.sync.dma_start(out=wf_t[:], in_=Wf[:])
            b0_t = pp.tile([F, 1], FDT)
            nc.sync.dma_start(out=b0_t[:], in_=b0[:])
            b1_t = pp.tile([F, 1], FDT)
            nc.sync.dma_start(out=b1_t[:], in_=b1[:])
            bf_t = pp.tile([F, 1], FDT)
            nc.sync.dma_start(out=bf_t[:], in_=bf[:])
            ident = pp.tile([P, P], FDT)
            make_identity(nc, ident[:])
            iota_col = pp.tile([P, P], HDT)
            nc.gpsimd.iota(
                iota_col[:], pattern=[[1, P]], base=0, channel_multiplier=0,
                allow_small_or_imprecise_dtypes=True,
            )
            iota_diag = pp.tile([P, P], HDT)
            nc.gpsimd.iota(
                iota_diag[:], pattern=[[1, P]], base=0, channel_multiplier=-1,
                allow_small_or_imprecise_dtypes=True,
            )
            xd_t = pp.tile([P, G * F], HDT, tag="xd")
            nc.sync.dma_start(out=xd_t[:], in_=xd_own[:])
            outt = pp.tile([P, G * F], FDT, tag="outt")

            def gather_layer(table_ap, scope):
                gts = [[] for _ in range(4)]
                with nc.named_scope(scope):
                    for k in range(int(ninst.max())):
                        for q in range(4):
                            if k >= ninst[q]:
                                continue
                            gt = gpools[q].tile([P, CPI, 4 * F], HDT, tag=f"gt{q}")
                            nc.gpsimd.dma_gather(
                                out_ap=gt[:],
                                in_ap=table_ap[q * QROWS:(q + 1) * QROWS, :],
                                idxs_ap=offs_t[
                                    :, scb[q] + k * (NIDX // 16)
                                    : scb[q] + (k + 1) * (NIDX // 16)
                                ],
                                num_idxs=NIDX,
                                num_idxs_reg=NIDX,
                                elem_size=4 * F,
                                queue_num=q,
                            )
                            gts[q].append(gt)
                return gts

            def consume_layer(gts, layer, scope):
                with nc.named_scope(scope):
                    col = 0
                    for t in range(G):
                        nchunks_t = int(segchunks[:, t].sum())
                        psa = psp.tile([F, P], FDT, tag="psa")
                        D = sp.tile([P, P], HDT, tag="S")
                        nc.vector.tensor_scalar(
                            out=D[:], in0=iota_diag[:], scalar1=0.0,
                            scalar2=dinv_t[:, t:t + 1],
                            op0=mybir.AluOpType.is_equal,
                            op1=mybir.AluOpType.mult,
                        )
                        nc.tensor.matmul(
                            out=psa[:], lhsT=xd_t[:, t * F:(t + 1) * F], rhs=D[:],
                            start=True, stop=(nchunks_t == 0),
                        )
                        done = 0
                        for q in range(4):
                            cb = int(tok_base[q, t]) // P
                            for i in range(int(segchunks[q, t])):
                                assert col == col_index[q, t] + i
                                k, s = divmod(cb + i, CPI)
                                S = sp.tile([P, P], HDT, tag="S")
                                nc.vector.tensor_scalar(
                                    out=S[:], in0=iota_col[:],
                                    scalar1=dstp_t[:, col:col + 1],
                                    scalar2=dinvc_t[:, col:col + 1],
                                    op0=mybir.AluOpType.is_equal,
                                    op1=mybir.AluOpType.mult,
                                )
                                done += 1
                                nc.tensor.matmul(
                                    out=psa[:], lhsT=gts[q][k][:, s, :F], rhs=S[:],
                                    start=False, stop=(done == nchunks_t),
                                )
                                col += 1
                        # tail (feature-major)
                        sa = fmp.tile([F, P], FDT, tag="sa")
                        nc.vector.tensor_copy(out=sa[:], in_=psa[:])
                        if layer == 0:
                            psb = psp.tile([F, P], FDT, tag="psb")
                            nc.tensor.matmul(
                                out=psb[:], lhsT=w0_t[:], rhs=sa[:],
                                start=True, stop=True,
                            )
                            sb = fmp.tile([F, P], FDT, tag="sb")
                            nc.scalar.activation(
                                out=sb[:], in_=psb[:],
                                func=mybir.ActivationFunctionType.Relu,
                                bias=b0_t[:, :1], scale=1.0,
                            )
                            psc = psp.tile([P, F], FDT, tag="psc")
                            nc.tensor.matmul(
                                out=psc[:], lhsT=sb[:], rhs=ident[:F, :F],
                                start=True, stop=True,
                            )
                            nc.vector.tensor_scalar(
                                out=xd_t[:, t * F:(t + 1) * F], in0=psc[:],
                                scalar1=dinv_t[:, t:t + 1], scalar2=None,
                                op0=mybir.AluOpType.mult,
                            )
                        else:
                            psb = psp.tile([F, P], FDT, tag="psb")
                            nc.tensor.matmul(
                                out=psb[:], lhsT=w1_t[:], rhs=sa[:],
                                start=True, stop=True,
                            )
                            sb = fmp.tile([F, P], FDT, tag="sb")
                            nc.scalar.activation(
                                out=sb[:], in_=psb[:],
                                func=mybir.ActivationFunctionType.Relu,
                                bias=b1_t[:, :1], scale=1.0,
                            )
                            psc = psp.tile([F, P], FDT, tag="psb")
                            nc.tensor.matmul(
                                out=psc[:], lhsT=wf_t[:], rhs=sb[:],
                                start=True, stop=True,
                            )
                            sc = fmp.tile([F, P], FDT, tag="sc")
                            nc.vector.tensor_scalar(
                                out=sc[:], in0=psc[:], scalar1=bf_t[:, :1],
                                scalar2=None, op0=mybir.AluOpType.add,
                            )
                            psd = psp.tile([P, F], FDT, tag="psc")
                            nc.tensor.matmul(
                                out=psd[:], lhsT=sc[:], rhs=ident[:F, :F],
                                start=True, stop=True,
                            )
                            nc.vector.tensor_copy(
                                out=outt[:, t * F:(t + 1) * F], in_=psd[:]
                            )

            gts0 = gather_layer(table0, "gather0")
            consume_layer(gts0, 0, "layer0")
            with nc.named_scope("table1"):
                nc.sync.dma_start(out=cc_in[:], in_=xd_t[:])
                nc.gpsimd.collective_compute(
                    "AllGather",
                    mybir.AluOpType.bypass,
                    replica_groups=[list(range(8))],
                    ins=[cc_in[:]],
                    outs=[cc_out[:]],
                )
                for qq in range(4):
                    nc.sync.dma_start(
                        out=table1[qq * QROWS:(qq + 1) * QROWS, :F],
                        in_=cc_out[qq * QROWS:(qq + 1) * QROWS, :],
                    )
            gts1 = gather_layer(table1, "gather1")
            consume_layer(gts1, 1, "layer1")
            nc.sync.dma_start(out=out_own[:], in_=outt[:])

    nc.compile()
    return nc


_CACHE = {}


def kernel(x, edge_index, W0, b0, W1, b1, Wf, bf):
    x = np.asarray(x, dtype=np.float32)
    edge_index = np.asarray(edge_index)
    plan, dinv, offs_all, dstp_all, dinvc_all = _build_plan(edge_index)

    key = ("prog", plan["OFFC"], plan["NCOLS"])
    if key not in _CACHE:
        _CACHE[key] = _build_program(plan)
    nc = _CACHE[key]

    # global fp16 table for layer 0: row (c*NPCP + p*G + t) = dinv*x of node
    # c*NPC + 128t + p
    nodes = np.arange(N)
    rank = nodes % NPC
    trow = (nodes // NPC) * NPCP + (rank % P) * G + (rank // P)
    tbl = np.zeros((NROWS, 4 * F), dtype=np.float16)
    tbl[trow, :F] = (x * dinv[:, None]).astype(np.float16)

    in_maps = []
    for c in range(8):
        lnodes = np.arange(c * NPC, (c + 1) * NPC)
        lrank = lnodes % NPC
        pp_, tt_ = lrank % P, lrank // P
        xd = np.zeros((P, G, F), dtype=np.float16)
        xd[pp_, tt_, :] = tbl[trow[lnodes], :F]
        dv = np.zeros((P, G), dtype=np.float32)
        dv[pp_, tt_] = dinv[lnodes]
        in_maps.append(
            {
                "table0": tbl,
                "xd_own": xd.reshape(P, G * F),
                "dinv_own": dv,
                "offs": np.tile(offs_all[c], (8, 1)).astype(np.int16),
                "dstp": dstp_all[c],
                "dinvc": dinvc_all[c],
                "W0": np.asarray(W0, np.float32),
                "W1": np.asarray(W1, np.float32),
                "Wf": np.asarray(Wf, np.float32),
                "b0": np.asarray(b0, np.float32).reshape(F, 1),
                "b1": np.asarray(b1, np.float32).reshape(F, 1),
                "bf": np.asarray(bf, np.float32).reshape(F, 1),
            }
        )

    res = run_bass_kernel_spmd(nc, in_maps, list(range(8)))
    kernel._last_results = res

    out = np.zeros((N, F), dtype=np.float32)
    for c in range(8):
        lnodes = np.arange(c * NPC, (c + 1) * NPC)
        lrank = lnodes % NPC
        pp_, tt_ = lrank % P, lrank // P
        oo = res.results[c]["out_own"].reshape(P, G, F)
        out[lnodes] = oo[pp_, tt_, :]
    return out
